# revision 12
# baseline (speedup 1.0000x reference)
"""Trainium2 Bass kernel for a single transformer decoder layer.

Reference semantics (B=64, T=200, E=512, H=8, D=64):
  x += SelfAttn(LN1(x))   (q,k row-masked by pred_mask, causal)
  x += CrossAttn(LN2(x))  (k from raw memory row-masked by src_mask,
                           v from LN2(x) (!), causal)
  x += FFN(LN3(x))        (512 -> 2048 -> relu -> 512)

Sharding: data-parallel over batch, 8 elems per NeuronCore, no collectives.

Layout strategy per batch element (all on one core):
  - residual stream x kept NATURAL [t_chunk<=128, 512] in fp32
  - LN via bn_stats/bn_aggr + two fused scalar_tensor_tensor ops
  - activations transposed to [E, T] via DMA-xbar transposes (bf16)
  - QK computed transposed [H*D, T] (lhsT = weights, reused stationaries)
  - scores computed TRANSPOSED  ST[s, t] = K Q^T  per head, 2 heads per
    PSUM bank; exp on ACT (no max subtraction -- scores are O(1));
    causal mask applied post-exp via gpsimd.affine_select(fill=0)
  - softmax denominators via matmuls with one-hot column stationaries
    into a [8, T] PSUM tile; normalization multiplied into O^T
  - AV gives O transposed directly (lhsT = V natural slices)
  - all biases enter PSUM via rank-1 (K=1) matmuls
"""

import numpy as np
import ml_dtypes
from contextlib import ExitStack

import concourse.bass as bass
import concourse.bacc as bacc
import concourse.tile as tile
from concourse import mybir
from concourse.bass_utils import run_bass_kernel_spmd

B, T, E, H, Dh, F = 64, 200, 512, 8, 64, 2048
NCORES = 8
SCALE = float(E) ** -0.5
F32 = mybir.dt.float32
BF16 = mybir.dt.bfloat16
AL = mybir.AluOpType
AF = mybir.ActivationFunctionType
TCH = [(0, 128), (128, 72)]  # token chunks (t0, tc)
ECH = E // 128  # 4
FCH = F // 128  # 16
NPBF16 = ml_dtypes.bfloat16

_programs = {}


def _layernorm(nc, pools, x_c, G, Bt, tc, eps):
    """x_c: [tc,512] f32 natural -> returns h_c [tc,512] bf16."""
    st6 = pools["small"].tile([tc, 6], F32, name="st6")
    nc.vector.bn_stats(st6[:, :], x_c)
    mv = pools["small"].tile([tc, 2], F32, name="mv")
    nc.vector.bn_aggr(mv[:, :], st6[:, :])
    std = pools["small"].tile([tc, 1], F32, name="std")
    nc.scalar.activation(std[:, :], mv[:, 1:2], AF.Sqrt, bias=eps[0:tc, 0:1])
    rstd = pools["small"].tile([tc, 1], F32, name="rstd")
    nc.vector.reciprocal(rstd[:, :], std[:, :])
    t1 = pools["lnt"].tile([tc, E], F32, name="t1")
    nc.vector.scalar_tensor_tensor(
        t1[:, :], x_c, mv[:, 0:1], G[0:tc, :], op0=AL.subtract, op1=AL.mult)
    h_c = pools["h"].tile([tc, E], BF16, name="h_c")
    if tc % 16:
        # zero the 32-aligned tail region covering the transpose pad rows
        # (CoreSim flags uninitialized reads; base partition must be 32-aligned)
        lo = 32 * (tc // 32)
        hi = 16 * (-(-(tc) // 16) + 1) if False else 16 * ((tc + 15) // 16)
        nc.gpsimd.memset(h_c.tensor[lo:hi, :], 0.0)
    nc.vector.scalar_tensor_tensor(
        h_c[:, :], t1[:, :], rstd[:, 0:1], Bt[0:tc, :], op0=AL.mult, op1=AL.add)
    return h_c


def _transpose_to(nc, pools, h_cs, dma_engine):
    """h_cs: [ [128,512], [72,512] ] bf16 natural -> hT[ec] [128,208] bf16
    (cols 0:200 valid) via DMA xbar transposes."""
    hT = []
    for ec in range(ECH):
        t = pools["tT"].tile([128, 208], BF16, name="hT", bufs=6)
        dma_engine.dma_start(
            t[:, 0:128], h_cs[0][0:128, ec * 128:(ec + 1) * 128], transpose=True)
        # second chunk has 72 rows; pad the read to 80 (within the padded
        # 128-partition tile) so p_dim % 16 == 0; cols 200:208 are garbage
        dma_engine.dma_start(
            t[:, 128:208], h_cs[1].tensor[0:80, ec * 128:(ec + 1) * 128],
            transpose=True)
        hT.append(t)
    return hT


def _attention(nc, pools, qT_sb, kT_sb, v_sb, sel_sb, selB, wo_sb, bo_row,
               ones_row, x_cs, out_dump=None):
    """Per-head causal attention + output projection + bias + residual.
    qT_sb/kT_sb: 4 tiles [128,200] bf16 ([2 heads * 64, T]);
    v_sb: 2 tiles [tc,512] bf16 natural; returns new residual tiles (f32)."""
    qT_lo, qT_hi = qT_sb
    kT_lo, kT_hi = kT_sb
    e0m, e1m = [], []
    # pass A: scores (transposed), exp, causal select; 2 heads per psum bank
    for oc in range(4):
        st0 = pools["ps_st"].tile([128, 2, 200], F32, name="st0", tag="st")
        st1 = pools["ps_st"].tile([72, 2, 72], F32, name="st1", tag="st")
        for hl in range(2):
            qh = qT_lo[oc][0:64, 0:200] if hl == 0 else qT_hi[oc][0:64, 0:200]
            kh = kT_lo[oc][0:64, 0:200] if hl == 0 else kT_hi[oc][0:64, 0:200]
            nc.tensor.matmul(st0[:, hl, :], kh[:, 0:128], qh)
            nc.tensor.matmul(st1[:, hl, :], kh[:, 128:200], qh[:, 128:200])
        e0 = pools["e0"].tile([128, 2, 200], BF16, name="e0", bufs=3)
        nc.scalar.activation(e0[:, :, :], st0[:, :, :], AF.Exp, scale=SCALE)
        e1 = pools["e1"].tile([72, 2, 72], BF16, name="e1", bufs=3)
        nc.scalar.activation(e1[:, :, :], st1[:, :, :], AF.Exp, scale=SCALE)
        # causal: keep where t - s >= 0 (iota = -p + t), else 0
        e0x = pools["e0"].tile([128, 2, 200], BF16, name="e0x", bufs=5)
        nc.gpsimd.affine_select(
            e0x[:, :, :], e0[:, :, :], pattern=[[0, 2], [1, 200]],
            compare_op=AL.is_ge, fill=0.0, base=0, channel_multiplier=-1)
        e1x = pools["e1"].tile([72, 2, 72], BF16, name="e1x", bufs=5)
        nc.gpsimd.affine_select(
            e1x[:, :, :], e1[:, :, :], pattern=[[0, 2], [1, 72]],
            compare_op=AL.is_ge, fill=0.0, base=0, channel_multiplier=-1)
        e0m.append(e0x)
        e1m.append(e1x)
    # pass B: denominators d[h, t] = sum_s exp -- one-hot stationaries
    dT = pools["ps_d"].tile([8, 200], F32, name="dT")
    for oc in range(4):
        for hl in range(2):
            h = 2 * oc + hl
            first = h == 0
            last = h == 7
            nc.tensor.matmul(dT[:, 0:200], sel_sb[0:128, h, :], e0m[oc][:, hl, :],
                             start=first, stop=False, skip_group_check=True)
            nc.tensor.matmul(dT[:, 128:200], sel_sb[0:72, h, :], e1m[oc][:, hl, :],
                             start=False, stop=last, skip_group_check=True)
    if out_dump is not None and out_dump[0] == 13:
        d, e = out_dump[1], out_dump[2]
        for oc in range(4):
            nc.gpsimd.dma_start(d[e, 0:128, oc * 128:(oc + 1) * 128],
                                e0m[oc][:, 0, 0:128])
        return x_cs
    dt_sb = pools["small"].tile([8, 200], F32, name="dt_sb")
    nc.vector.tensor_copy(dt_sb[:, :], dT[:, :])
    dinvT = pools["small"].tile([8, 200], F32, name="dinvT")
    nc.vector.reciprocal_approx_fast(dinvT[:, :], dt_sb[:, :])
    # pass C: O^T = V^T @ E^T, then normalize by 1/d broadcast to the two
    # 64-partition head halves via a one-hot matmul (selB)
    oT_sb = []
    for oc in range(4):
        dbc_ps = pools["ps_ot"].tile([128, 200], F32, name="dbc_ps", tag="ot_ps")
        nc.tensor.matmul(dbc_ps[:, :], selB[0:8, oc, :], dinvT[:, :])
        dbc = pools["dbc"].tile([128, 200], F32, name="dbc")
        nc.scalar.copy(dbc[:, :], dbc_ps[:, :])
        ot_ps = pools["ps_ot"].tile([128, 200], F32, name="ot_ps", tag="ot_ps")
        for hl in range(2):
            h = 2 * oc + hl
            hp = hl * 64
            nc.tensor.matmul(ot_ps[hp:hp + 64, 0:200],
                             v_sb[0][0:128, h * 64:(h + 1) * 64],
                             e0m[oc][:, hl, :], start=True, stop=False,
                             skip_group_check=True)
            nc.tensor.matmul(ot_ps[hp:hp + 64, 128:200],
                             v_sb[1][0:72, h * 64:(h + 1) * 64],
                             e1m[oc][:, hl, :], start=False, stop=True,
                             skip_group_check=True)
        ot = pools["ot"].tile([128, 200], BF16, name="ot")
        nc.vector.tensor_mul(ot[:, :], ot_ps[:, :], dbc[:, :])
        oT_sb.append(ot)
    if out_dump is not None and out_dump[0] == 14:
        d, e = out_dump[1], out_dump[2]
        for oc in range(4):
            nc.gpsimd.dma_start(d[e, 0:128, oc * 128:(oc + 1) * 128],
                                oT_sb[oc][:, 0:128])
        return x_cs
    # output projection (natural out) + bias via rank-1 matmul + residual
    new_x = []
    for ci, (t0, tc) in enumerate(TCH):
        ps = pools["ps_proj"].tile([tc, E], F32, name="proj_ps", tag="pp")
        for hc in range(4):
            nc.tensor.matmul(ps[:, :], oT_sb[hc][:, t0:t0 + tc],
                             wo_sb[:, hc, :], start=(hc == 0), stop=False)
        nc.tensor.matmul(ps[:, :], ones_row[0:1, 0:tc], bo_row[0:1, :],
                         start=False, stop=True)
        xn = pools["res"].tile([tc, E], F32, name="xn", tag="res")
        nc.vector.tensor_add(xn[:, :], ps[:, :], x_cs[ci])
        new_x.append(xn)
    return new_x


def _project_qkT(nc, pools, w_sb, rhs_T, name):
    """out[oc] [128,200] bf16 = (W^T h)^T chunks; w_sb [128,4,512] bf16,
    rhs_T: 4 tiles [128,208] (cols 0:200). Also returns a base-partition-0
    copy of rows 64:128 per chunk (matmul operands at base partition 64
    are rejected/crash, so odd heads must read from partition 0)."""
    out, hi = [], []
    for oc in range(4):
        ps = pools["ps_proj"].tile([128, 200], F32, name=f"{name}_ps", tag="pp")
        for ec in range(ECH):
            nc.tensor.matmul(ps[:, :], w_sb[:, ec, oc * 128:(oc + 1) * 128],
                             rhs_T[ec][:, 0:200], start=(ec == 0), stop=(ec == 3))
        sb = pools["qkt"].tile([128, 200], BF16, name=f"{name}_sb")
        nc.vector.tensor_copy(sb[:, :], ps[:, :])
        hb = pools["qkt"].tile([64, 200], BF16, name=f"{name}_hi", tag="hi5")
        nc.scalar.dma_start(hb[:, :], sb[64:128, :])
        out.append(sb)
        hi.append(hb)
    return out, hi


def _project_v(nc, pools, wv_sb, hT, name):
    """v natural [tc, 512] bf16 tiles (lhsT = hT slices, rhs = Wv)."""
    out = []
    for (t0, tc) in TCH:
        ps = pools["ps_proj"].tile([tc, E], F32, name=f"{name}_ps", tag="pp")
        for ec in range(ECH):
            nc.tensor.matmul(ps[:, :], hT[ec][:, t0:t0 + tc], wv_sb[:, ec, :],
                             start=(ec == 0), stop=(ec == 3))
        sb = pools["v"].tile([tc, E], BF16, name=f"{name}_sb")
        nc.scalar.copy(sb[:, :], ps[:, :])
        out.append(sb)
    return out


def _build(bpc, stages=3):
    nc = bacc.Bacc("TRN2", target_bir_lowering=False, debug=False,
                   enable_asserts=False, num_devices=NCORES)
    dram = {}

    def din(name, shape, dt):
        h = nc.dram_tensor(name, list(shape), dt, kind="ExternalInput")
        dram[name] = h
        return h

    x_d = din("x", (bpc, T, E), F32)
    mem_d = din("mem", (bpc, T, E), BF16)
    pm_d = din("pm", (bpc, T), BF16)
    sm_d = din("sm", (bpc, T), BF16)
    wq_sa_d = din("wq_sa", (E, E), BF16)
    wk_sa_d = din("wk_sa", (E, E), BF16)
    wv_sa_d = din("wv_sa", (E, E), BF16)
    wo_sa_d = din("wo_sa", (E, E), BF16)
    bo_sa_d = din("bo_sa", (1, E), BF16)
    wq_ca_d = din("wq_ca", (E, E), BF16)
    wk_ca_d = din("wk_ca", (E, E), BF16)
    wv_ca_d = din("wv_ca", (E, E), BF16)
    wo_ca_d = din("wo_ca", (E, E), BF16)
    bo_ca_d = din("bo_ca", (1, E), BF16)
    w1_d = din("w1", (E, F), BF16)
    b1_d = din("b1", (1, F), BF16)
    w2_d = din("w2", (F, E), BF16)
    b2_d = din("b2", (1, E), BF16)
    gb_d = {k: din(k, (1, E), F32)
            for k in ("g1", "be1", "g2", "be2", "g3", "be3")}
    out_d = nc.dram_tensor("out", [bpc, T, E], F32, kind="ExternalOutput")

    sel_np = np.zeros((128, 8, 8), dtype=NPBF16)
    for h in range(8):
        sel_np[:, h, h] = 1
    sel_d = nc.inline_tensor(sel_np, name="selc")
    ones_d = nc.inline_tensor(np.ones((1, E), dtype=NPBF16), name="onesc")
    selB_np = np.zeros((8, 4, 128), dtype=np.float32)
    for oc in range(4):
        selB_np[2 * oc, oc, 0:64] = 1
        selB_np[2 * oc + 1, oc, 64:128] = 1
    selB_d = nc.inline_tensor(selB_np, name="selBc")

    with tile.TileContext(nc) as tc_ctx, ExitStack() as ctx:
        tcx = tc_ctx
        pools = {}

        def pool(name, bufs, space="SBUF"):
            pools[name] = ctx.enter_context(
                tcx.tile_pool(name=name, bufs=bufs, space=space))
            return pools[name]

        wpool = pool("w", 1)
        pool("small", 6)
        pool("lnt", 3)
        pool("h", 5)
        pool("tT", 3)
        pool("qkt", 5)
        pool("v", 3)
        pool("e0", 3)
        pool("e1", 3)
        pool("ot", 6)
        pool("dbc", 3)
        pool("res", 8)
        pool("rT", 10)
        pool("mrow", 3)
        pool("mbc", 4)
        pool("ps_proj", 3, space="PSUM")
        pool("ps_st", 2, space="PSUM")
        pool("ps_d", 1, space="PSUM")
        pool("ps_ot", 2, space="PSUM")

        def wtile(name, src, shape, rearr=None, dt=BF16):
            t = wpool.tile(shape, dt, tag=name, bufs=1, name=name)
            ap = src[:] if rearr is None else src[:].rearrange(rearr, p=128)
            nc.sync.dma_start(t[...], ap)
            return t

        wq_sa = wtile("wq_sa", wq_sa_d, [128, ECH, E], "(c p) n -> p c n")
        wk_sa = wtile("wk_sa", wk_sa_d, [128, ECH, E], "(c p) n -> p c n")
        wv_sa = wtile("wv_sa", wv_sa_d, [128, ECH, E], "(c p) n -> p c n")
        wo_sa = wtile("wo_sa", wo_sa_d, [128, ECH, E], "(c p) n -> p c n")
        wq_ca = wtile("wq_ca", wq_ca_d, [128, ECH, E], "(c p) n -> p c n")
        wk_ca = wtile("wk_ca", wk_ca_d, [128, ECH, E], "(c p) n -> p c n")
        wv_ca = wtile("wv_ca", wv_ca_d, [128, ECH, E], "(c p) n -> p c n")
        wo_ca = wtile("wo_ca", wo_ca_d, [128, ECH, E], "(c p) n -> p c n")
        w1 = wtile("w1", w1_d, [128, ECH, F], "(c p) n -> p c n")
        w2 = wtile("w2", w2_d, [128, FCH, E], "(c p) n -> p c n")
        bo_sa = wtile("bo_sa", bo_sa_d, [1, E])
        bo_ca = wtile("bo_ca", bo_ca_d, [1, E])
        b1r = wtile("b1", b1_d, [1, F])
        b2r = wtile("b2", b2_d, [1, E])
        sel_sb = wtile("sel", sel_d, [128, 8, 8])
        selB = wtile("selB", selB_d, [8, 4, 128], dt=F32)
        ones_row = wtile("ones", ones_d, [1, E])

        gb = {}
        for k in ("g1", "be1", "g2", "be2", "g3", "be3"):
            row = pools["mrow"].tile([1, E], F32, tag="gbrow", bufs=2, name=k + "r")
            nc.sync.dma_start(row[...], gb_d[k][:])
            bc = wpool.tile([128, E], F32, tag=k, bufs=1, name=k)
            nc.gpsimd.partition_broadcast(bc[:, :], row[:, :])
            gb[k] = bc
        eps = wpool.tile([128, 1], F32, tag="eps", bufs=1, name="eps")
        nc.gpsimd.memset(eps[:, :], 1e-5)

        for e in range(bpc):
            # ---- load x, masks ----
            x_cs = []
            for (t0, tc) in TCH:
                xt = pools["res"].tile([tc, E], F32, name="x_in", tag="res")
                nc.sync.dma_start(xt[:, :], x_d[e, t0:t0 + tc, :])
                x_cs.append(xt)
            pm_row = pools["mrow"].tile([1, T], BF16, name="pm_row")
            nc.sync.dma_start(pm_row[:, :], pm_d[e:e + 1, :])
            pm_bc = pools["mbc"].tile([128, T], BF16, name="pm_bc")
            nc.gpsimd.partition_broadcast(pm_bc[:, :], pm_row[:, :])
            sm_row = pools["mrow"].tile([1, T], BF16, name="sm_row")
            nc.sync.dma_start(sm_row[:, :], sm_d[e:e + 1, :])
            sm_bc = pools["mbc"].tile([128, T], BF16, name="sm_bc")
            nc.gpsimd.partition_broadcast(sm_bc[:, :], sm_row[:, :])

            # ======== self-attention ========
            h_cs = [_layernorm(nc, pools, x_cs[ci][:, :], gb["g1"], gb["be1"], tc, eps)
                    for ci, (t0, tc) in enumerate(TCH)]
            if stages == 10:
                for ci, (t0, tc) in enumerate(TCH):
                    nc.gpsimd.dma_start(out_d[e, t0:t0 + tc, :], h_cs[ci][:, :])
                continue
            hT = _transpose_to(nc, pools, h_cs, nc.sync)
            if stages == 11:
                for ec in range(ECH):
                    nc.gpsimd.dma_start(out_d[e, 0:128, ec * 128:(ec + 1) * 128],
                                        hT[ec][:, 0:128])
                continue
            # masked (pred) transposed h for q,k
            hmT = []
            for ec in range(ECH):
                m = pools["tT"].tile([128, 208], BF16, name="hmT", bufs=5)
                nc.vector.tensor_mul(m[:, 0:200], hT[ec][:, 0:200], pm_bc[:, :])
                hmT.append(m)
            qT = _project_qkT(nc, pools, wq_sa, hmT, "q_sa")
            kT = _project_qkT(nc, pools, wk_sa, hmT, "k_sa")
            v_sb = _project_v(nc, pools, wv_sa, hT, "v_sa")
            if stages == 12:
                for ci, (t0, tc) in enumerate(TCH):
                    nc.gpsimd.dma_start(out_d[e, t0:t0 + tc, :], v_sb[ci][:, :])
                for oc in range(4):
                    nc.gpsimd.dma_start(out_d[e, 0:128, oc * 128:(oc + 1) * 128],
                                        qT[oc][:, 0:128])
                    nc.gpsimd.dma_start(out_d[e, 128:200, oc * 128:(oc + 1) * 128],
                                        kT[oc][0:72, 0:128])
                continue
            x_cs = _attention(nc, pools, qT, kT, v_sb, sel_sb, selB, wo_sa,
                              bo_sa, ones_row, x_cs,
                              out_dump=(stages, out_d, e) if stages in (13, 14) else None)
            if stages in (13, 14):
                continue
            if stages == 1:
                for ci, (t0, tc) in enumerate(TCH):
                    nc.sync.dma_start(out_d[e, t0:t0 + tc, :], x_cs[ci][:, :])
                continue

            # ======== cross-attention ========
            h_cs = [_layernorm(nc, pools, x_cs[ci][:, :], gb["g2"], gb["be2"], tc, eps)
                    for ci, (t0, tc) in enumerate(TCH)]
            h2T = _transpose_to(nc, pools, h_cs, nc.scalar)
            # memory transposed straight from DRAM (two overlapping reads)
            memT = []
            for ec in range(ECH):
                t = pools["tT"].tile([128, 208], BF16, name="memT", bufs=3)
                nc.sync.dma_start(t[:, 0:128],
                                  mem_d[e, 0:128, ec * 128:(ec + 1) * 128],
                                  transpose=True)
                nc.sync.dma_start(t[:, 72:200],
                                  mem_d[e, 72:200, ec * 128:(ec + 1) * 128],
                                  transpose=True)
                mm = pools["tT"].tile([128, 208], BF16, name="memTm", bufs=5)
                nc.vector.tensor_mul(mm[:, 0:200], t[:, 0:200], sm_bc[:, :])
                memT.append(mm)
            qT = _project_qkT(nc, pools, wq_ca, h2T, "q_ca")
            kT = _project_qkT(nc, pools, wk_ca, memT, "k_ca")
            v_sb = _project_v(nc, pools, wv_ca, h2T, "v_ca")
            x_cs = _attention(nc, pools, qT, kT, v_sb, sel_sb, selB, wo_ca,
                              bo_ca, ones_row, x_cs)
            if stages == 2:
                for ci, (t0, tc) in enumerate(TCH):
                    nc.sync.dma_start(out_d[e, t0:t0 + tc, :], x_cs[ci][:, :])
                continue

            # ======== feed-forward ========
            h_cs = [_layernorm(nc, pools, x_cs[ci][:, :], gb["g3"], gb["be3"], tc, eps)
                    for ci, (t0, tc) in enumerate(TCH)]
            h3T = _transpose_to(nc, pools, h_cs, nc.scalar)
            rT = []
            for fcp in range(FCH // 2):
                zps = pools["ps_st"].tile([128, 2, 200], F32, name="z_ps", tag="st")
                for fl in range(2):
                    fc = 2 * fcp + fl
                    for ec in range(ECH):
                        nc.tensor.matmul(
                            zps[:, fl, :], w1[:, ec, fc * 128:(fc + 1) * 128],
                            h3T[ec][:, 0:200], start=(ec == 0), stop=False)
                    nc.tensor.matmul(
                        zps[:, fl, :], b1r[0:1, fc * 128:(fc + 1) * 128],
                        ones_row[0:1, 0:200], start=False, stop=True)
                r = pools["rT"].tile([128, 2, 200], BF16, name="r")
                nc.scalar.activation(r[:, :, :], zps[:, :, :], AF.Relu)
                rT.append(r)
            for ci, (t0, tc) in enumerate(TCH):
                yps = pools["ps_proj"].tile([tc, E], F32, name="y_ps", tag="pp")
                for fcp in range(FCH // 2):
                    for fl in range(2):
                        fc = 2 * fcp + fl
                        nc.tensor.matmul(yps[:, :], rT[fcp][:, fl, t0:t0 + tc],
                                         w2[:, fc, :], start=(fc == 0), stop=False)
                nc.tensor.matmul(yps[:, :], ones_row[0:1, 0:tc], b2r[0:1, :],
                                 start=False, stop=True)
                yout = pools["res"].tile([tc, E], F32, name="yout", tag="res")
                nc.vector.tensor_add(yout[:, :], yps[:, :], x_cs[ci][:, :])
                nc.sync.dma_start(out_d[e, t0:t0 + tc, :], yout[:, :])

    nc.compile()
    return nc


def _host_prep(inputs, bpc, core):
    """Build the in_map for one core."""
    s = slice(core * bpc, (core + 1) * bpc)

    def rearr(w):  # (H, E, D) -> [E, H*D]
        return np.ascontiguousarray(
            np.transpose(np.asarray(w, np.float32), (1, 0, 2)).reshape(E, E)
        ).astype(NPBF16)

    def b16(a):
        return np.ascontiguousarray(np.asarray(a, np.float32)).astype(NPBF16)

    def f32c(a):
        return np.ascontiguousarray(np.asarray(a, np.float32))

    return {
        "x": f32c(inputs["idx"][s]),
        "mem": b16(inputs["memory"][s]),
        "pm": b16(inputs["pred_mask"][s] != 0),
        "sm": b16(inputs["src_mask"][s] != 0),
        "wq_sa": rearr(inputs["sa_wq"]), "wk_sa": rearr(inputs["sa_wk"]),
        "wv_sa": rearr(inputs["sa_wv"]),
        "wo_sa": b16(inputs["sa_wo"]), "bo_sa": b16(inputs["sa_bo"]).reshape(1, E),
        "wq_ca": rearr(inputs["ca_wq"]), "wk_ca": rearr(inputs["ca_wk"]),
        "wv_ca": rearr(inputs["ca_wv"]),
        "wo_ca": b16(inputs["ca_wo"]), "bo_ca": b16(inputs["ca_bo"]).reshape(1, E),
        "w1": b16(inputs["f_w1"]), "b1": b16(inputs["f_b1"]).reshape(1, F),
        "w2": b16(inputs["f_w2"]), "b2": b16(inputs["f_b2"]).reshape(1, E),
        "g1": f32c(inputs["ln1_g"]).reshape(1, E),
        "be1": f32c(inputs["ln1_b"]).reshape(1, E),
        "g2": f32c(inputs["ln2_g"]).reshape(1, E),
        "be2": f32c(inputs["ln2_b"]).reshape(1, E),
        "g3": f32c(inputs["ln3_g"]).reshape(1, E),
        "be3": f32c(inputs["ln3_b"]).reshape(1, E),
    }


def get_program(bpc):
    if bpc not in _programs:
        _programs[bpc] = _build(bpc)
    return _programs[bpc]


def kernel(**inputs) -> np.ndarray:
    bpc = B // NCORES
    nc = get_program(bpc)
    in_maps = [_host_prep(inputs, bpc, c) for c in range(NCORES)]
    res = run_bass_kernel_spmd(nc, in_maps, core_ids=list(range(NCORES)))
    out = np.concatenate([res.results[c]["out"] for c in range(NCORES)], axis=0)
    return out.astype(np.float32)


# revision 14
# speedup vs baseline: 1.3193x; 1.3193x over previous
"""Trainium2 Bass kernel for a single transformer decoder layer.

Reference semantics (B=64, T=200, E=512, H=8, D=64):
  x += SelfAttn(LN1(x))   (q,k row-masked by pred_mask, causal)
  x += CrossAttn(LN2(x))  (k from raw memory row-masked by src_mask,
                           v from LN2(x) (!), causal)
  x += FFN(LN3(x))        (512 -> 2048 -> relu -> 512)

Sharding: data-parallel over batch, 8 elems per NeuronCore, no collectives.

Layout strategy per batch element (all on one core):
  - residual stream x kept NATURAL [t_chunk<=128, 512] in fp32
  - LN via bn_stats/bn_aggr + two fused scalar_tensor_tensor ops
  - activations transposed to [E, T] via DMA-xbar transposes (bf16)
  - QK computed transposed [H*D, T] (lhsT = weights, reused stationaries)
  - scores computed TRANSPOSED  ST[s, t] = K Q^T  per head, 2 heads per
    PSUM bank; exp on ACT (no max subtraction -- scores are O(1));
    causal mask applied post-exp via gpsimd.affine_select(fill=0)
  - softmax denominators via matmuls with one-hot column stationaries
    into a [8, T] PSUM tile; normalization multiplied into O^T
  - AV gives O transposed directly (lhsT = V natural slices)
  - all biases enter PSUM via rank-1 (K=1) matmuls
"""

import numpy as np
import ml_dtypes
from contextlib import ExitStack

import concourse.bass as bass
import concourse.bacc as bacc
import concourse.tile as tile
from concourse import mybir
from concourse.bass_utils import run_bass_kernel_spmd

B, T, E, H, Dh, F = 64, 200, 512, 8, 64, 2048
NCORES = 8
SCALE = float(E) ** -0.5
F32 = mybir.dt.float32
BF16 = mybir.dt.bfloat16
AL = mybir.AluOpType
AF = mybir.ActivationFunctionType
TCH = [(0, 128), (128, 72)]  # token chunks (t0, tc)
ECH = E // 128  # 4
FCH = F // 128  # 16
NPBF16 = ml_dtypes.bfloat16

_programs = {}


def _layernorm(nc, pools, x_c, G, Bt, tc, eps):
    """x_c: [tc,512] f32 natural -> returns h_c [tc,512] bf16."""
    st6 = pools["small"].tile([tc, 6], F32, name="st6")
    nc.vector.bn_stats(st6[:, :], x_c)
    mv = pools["small"].tile([tc, 2], F32, name="mv")
    nc.vector.bn_aggr(mv[:, :], st6[:, :])
    std = pools["small"].tile([tc, 1], F32, name="std")
    nc.scalar.activation(std[:, :], mv[:, 1:2], AF.Sqrt, bias=eps[0:tc, 0:1])
    rstd = pools["small"].tile([tc, 1], F32, name="rstd")
    nc.vector.reciprocal(rstd[:, :], std[:, :])
    t1 = pools["lnt"].tile([tc, E], F32, name="t1")
    nc.vector.scalar_tensor_tensor(
        t1[:, :], x_c, mv[:, 0:1], G[0:tc, :], op0=AL.subtract, op1=AL.mult)
    h_c = pools["h"].tile([tc, E], BF16, name="h_c")
    nc.vector.scalar_tensor_tensor(
        h_c[:, :], t1[:, :], rstd[:, 0:1], Bt[0:tc, :], op0=AL.mult, op1=AL.add)
    return h_c


def _transpose_to(nc, pools, h_cs, ident):
    """h_cs: [ [128,512], [72,512] ] bf16 natural -> hT[ec] [128,200] bf16
    via PE is_transpose matmuls (keeps the PE warm; DVE drains PSUM)."""
    hT = []
    for ec in range(ECH):
        t = pools["tT"].tile([128, 200], BF16, name="hT", bufs=6)
        for ci, (t0, tc) in enumerate(TCH):
            ps = pools["ps_proj"].tile([128, tc], BF16, name="t_ps", tag="pp")
            nc.tensor.transpose(ps[:, :], h_cs[ci][0:tc, ec * 128:(ec + 1) * 128],
                                ident[0:tc, 0:tc])
            nc.vector.tensor_copy(t[:, t0:t0 + tc], ps[:, :])
        hT.append(t)
    return hT


def _attention(nc, pools, qT_sb, kT_sb, v_sb, sel_sb, selB, wo_sb, bo_row,
               ones_row, x_cs, out_dump=None):
    """Per-head causal attention + output projection + bias + residual.
    qT_sb/kT_sb: 4 tiles [128,200] bf16 ([2 heads * 64, T]);
    v_sb: 2 tiles [tc,512] bf16 natural; returns new residual tiles (f32)."""
    qT_lo, qT_hi = qT_sb
    kT_lo, kT_hi = kT_sb
    e0m, e1m = [], []
    # pass A: scores (transposed), exp, causal select; 2 heads per psum bank
    for oc in range(4):
        st0 = pools["ps_st"].tile([128, 2, 200], F32, name="st0", tag="st")
        st1 = pools["ps_st"].tile([72, 2, 72], F32, name="st1", tag="st")
        for hl in range(2):
            qh = qT_lo[oc][0:64, 0:200] if hl == 0 else qT_hi[oc][0:64, 0:200]
            kh = kT_lo[oc][0:64, 0:200] if hl == 0 else kT_hi[oc][0:64, 0:200]
            nc.tensor.matmul(st0[:, hl, :], kh[:, 0:128], qh)
            nc.tensor.matmul(st1[:, hl, :], kh[:, 128:200], qh[:, 128:200])
        e0 = pools["e0"].tile([128, 2, 200], BF16, name="e0", bufs=3)
        nc.scalar.activation(e0[:, :, :], st0[:, :, :], AF.Exp, scale=SCALE)
        e1 = pools["e1"].tile([72, 2, 72], BF16, name="e1", bufs=3)
        nc.scalar.activation(e1[:, :, :], st1[:, :, :], AF.Exp, scale=SCALE)
        # causal: keep where t - s >= 0 (iota = -p + t), else 0
        e0x = pools["e0"].tile([128, 2, 200], BF16, name="e0x", bufs=5)
        nc.gpsimd.affine_select(
            e0x[:, :, :], e0[:, :, :], pattern=[[0, 2], [1, 200]],
            compare_op=AL.is_ge, fill=0.0, base=0, channel_multiplier=-1)
        e1x = pools["e1"].tile([72, 2, 72], BF16, name="e1x", bufs=5)
        nc.gpsimd.affine_select(
            e1x[:, :, :], e1[:, :, :], pattern=[[0, 2], [1, 72]],
            compare_op=AL.is_ge, fill=0.0, base=0, channel_multiplier=-1)
        e0m.append(e0x)
        e1m.append(e1x)
    # pass B: denominators d[h, t] = sum_s exp -- one-hot stationaries
    dT = pools["ps_d"].tile([8, 200], F32, name="dT")
    for oc in range(4):
        for hl in range(2):
            h = 2 * oc + hl
            first = h == 0
            last = h == 7
            nc.tensor.matmul(dT[:, 0:200], sel_sb[0:128, h, :], e0m[oc][:, hl, :],
                             start=first, stop=False, skip_group_check=True)
            nc.tensor.matmul(dT[:, 128:200], sel_sb[0:72, h, :], e1m[oc][:, hl, :],
                             start=False, stop=last, skip_group_check=True)
    if out_dump is not None and out_dump[0] == 13:
        d, e = out_dump[1], out_dump[2]
        for oc in range(4):
            nc.gpsimd.dma_start(d[e, 0:128, oc * 128:(oc + 1) * 128],
                                e0m[oc][:, 0, 0:128])
        return x_cs
    dt_sb = pools["small"].tile([8, 200], F32, name="dt_sb")
    nc.vector.tensor_copy(dt_sb[:, :], dT[:, :])
    dinvT = pools["small"].tile([8, 200], F32, name="dinvT")
    nc.vector.reciprocal_approx_fast(dinvT[:, :], dt_sb[:, :])
    # pass C: O^T = V^T @ E^T, then normalize by 1/d broadcast to the two
    # 64-partition head halves via a one-hot matmul (selB)
    oT_sb = []
    for oc in range(4):
        dbc_ps = pools["ps_ot"].tile([128, 200], F32, name="dbc_ps", tag="ot_ps")
        nc.tensor.matmul(dbc_ps[:, :], selB[0:8, oc, :], dinvT[:, :])
        dbc = pools["dbc"].tile([128, 200], F32, name="dbc")
        nc.vector.tensor_copy(dbc[:, :], dbc_ps[:, :])
        ot_ps = pools["ps_ot"].tile([128, 200], F32, name="ot_ps", tag="ot_ps")
        for hl in range(2):
            h = 2 * oc + hl
            hp = hl * 64
            nc.tensor.matmul(ot_ps[hp:hp + 64, 0:200],
                             v_sb[0][0:128, h * 64:(h + 1) * 64],
                             e0m[oc][:, hl, :], start=True, stop=False,
                             skip_group_check=True)
            nc.tensor.matmul(ot_ps[hp:hp + 64, 128:200],
                             v_sb[1][0:72, h * 64:(h + 1) * 64],
                             e1m[oc][:, hl, :], start=False, stop=True,
                             skip_group_check=True)
        ot = pools["ot"].tile([128, 200], BF16, name="ot")
        nc.vector.tensor_mul(ot[:, :], ot_ps[:, :], dbc[:, :])
        oT_sb.append(ot)
    if out_dump is not None and out_dump[0] == 14:
        d, e = out_dump[1], out_dump[2]
        for oc in range(4):
            nc.gpsimd.dma_start(d[e, 0:128, oc * 128:(oc + 1) * 128],
                                oT_sb[oc][:, 0:128])
        return x_cs
    # output projection (natural out) + bias via rank-1 matmul + residual
    new_x = []
    for ci, (t0, tc) in enumerate(TCH):
        ps = pools["ps_proj"].tile([tc, E], F32, name="proj_ps", tag="pp")
        for hc in range(4):
            nc.tensor.matmul(ps[:, :], oT_sb[hc][:, t0:t0 + tc],
                             wo_sb[:, hc, :], start=(hc == 0), stop=False)
        nc.tensor.matmul(ps[:, :], ones_row[0:1, 0:tc], bo_row[0:1, :],
                         start=False, stop=True)
        xn = pools["res"].tile([tc, E], F32, name="xn", tag="res")
        nc.vector.tensor_add(xn[:, :], ps[:, :], x_cs[ci])
        new_x.append(xn)
    return new_x


def _project_qkT(nc, pools, w_sb, rhs_T, name):
    """out[oc] [128,200] bf16 = (W^T h)^T chunks; w_sb [128,4,512] bf16,
    rhs_T: 4 tiles [128,208] (cols 0:200). Also returns a base-partition-0
    copy of rows 64:128 per chunk (matmul operands at base partition 64
    are rejected/crash, so odd heads must read from partition 0)."""
    out, hi = [], []
    for oc in range(4):
        ps = pools["ps_proj"].tile([128, 200], F32, name=f"{name}_ps", tag="pp")
        for ec in range(ECH):
            nc.tensor.matmul(ps[:, :], w_sb[:, ec, oc * 128:(oc + 1) * 128],
                             rhs_T[ec][:, 0:200], start=(ec == 0), stop=(ec == 3))
        sb = pools["qkt"].tile([128, 200], BF16, name=f"{name}_sb")
        nc.vector.tensor_copy(sb[:, :], ps[:, :])
        hb = pools["qkt"].tile([64, 200], BF16, name=f"{name}_hi", tag="hi5")
        nc.scalar.dma_start(hb[:, :], sb[64:128, :])
        out.append(sb)
        hi.append(hb)
    return out, hi


def _project_v(nc, pools, wv_sb, hT, name):
    """v natural [tc, 512] bf16 tiles (lhsT = hT slices, rhs = Wv)."""
    out = []
    for (t0, tc) in TCH:
        ps = pools["ps_proj"].tile([tc, E], F32, name=f"{name}_ps", tag="pp")
        for ec in range(ECH):
            nc.tensor.matmul(ps[:, :], hT[ec][:, t0:t0 + tc], wv_sb[:, ec, :],
                             start=(ec == 0), stop=(ec == 3))
        sb = pools["v"].tile([tc, E], BF16, name=f"{name}_sb")
        nc.scalar.copy(sb[:, :], ps[:, :])
        out.append(sb)
    return out


def _build(bpc, stages=3):
    nc = bacc.Bacc("TRN2", target_bir_lowering=False, debug=False,
                   enable_asserts=False, num_devices=NCORES)
    dram = {}

    def din(name, shape, dt):
        h = nc.dram_tensor(name, list(shape), dt, kind="ExternalInput")
        dram[name] = h
        return h

    x_d = din("x", (bpc, T, E), F32)
    mem_d = din("mem", (bpc, T, E), BF16)
    pm_d = din("pm", (bpc, T), BF16)
    sm_d = din("sm", (bpc, T), BF16)
    wq_sa_d = din("wq_sa", (E, E), BF16)
    wk_sa_d = din("wk_sa", (E, E), BF16)
    wv_sa_d = din("wv_sa", (E, E), BF16)
    wo_sa_d = din("wo_sa", (E, E), BF16)
    bo_sa_d = din("bo_sa", (1, E), BF16)
    wq_ca_d = din("wq_ca", (E, E), BF16)
    wk_ca_d = din("wk_ca", (E, E), BF16)
    wv_ca_d = din("wv_ca", (E, E), BF16)
    wo_ca_d = din("wo_ca", (E, E), BF16)
    bo_ca_d = din("bo_ca", (1, E), BF16)
    w1_d = din("w1", (E, F), BF16)
    b1_d = din("b1", (1, F), BF16)
    w2_d = din("w2", (F, E), BF16)
    b2_d = din("b2", (1, E), BF16)
    gb_d = {k: din(k, (1, E), F32)
            for k in ("g1", "be1", "g2", "be2", "g3", "be3")}
    out_d = nc.dram_tensor("out", [bpc, T, E], F32, kind="ExternalOutput")

    sel_np = np.zeros((128, 8, 8), dtype=NPBF16)
    for h in range(8):
        sel_np[:, h, h] = 1
    sel_d = nc.inline_tensor(sel_np, name="selc")
    ones_d = nc.inline_tensor(np.ones((1, E), dtype=NPBF16), name="onesc")
    selB_np = np.zeros((8, 4, 128), dtype=np.float32)
    for oc in range(4):
        selB_np[2 * oc, oc, 0:64] = 1
        selB_np[2 * oc + 1, oc, 64:128] = 1
    selB_d = nc.inline_tensor(selB_np, name="selBc")
    identb_d = nc.inline_tensor(np.eye(128, dtype=NPBF16), name="identbc")

    with tile.TileContext(nc) as tc_ctx, ExitStack() as ctx:
        tcx = tc_ctx
        pools = {}

        def pool(name, bufs, space="SBUF"):
            pools[name] = ctx.enter_context(
                tcx.tile_pool(name=name, bufs=bufs, space=space))
            return pools[name]

        wpool = pool("w", 1)
        pool("small", 6)
        pool("lnt", 3)
        pool("h", 5)
        pool("tT", 3)
        pool("qkt", 5)
        pool("v", 3)
        pool("e0", 3)
        pool("e1", 3)
        pool("ot", 6)
        pool("dbc", 3)
        pool("res", 8)
        pool("rT", 10)
        pool("mrow", 3)
        pool("mbc", 4)
        pool("ps_proj", 3, space="PSUM")
        pool("ps_st", 2, space="PSUM")
        pool("ps_d", 1, space="PSUM")
        pool("ps_ot", 2, space="PSUM")

        def wtile(name, src, shape, rearr=None, dt=BF16):
            t = wpool.tile(shape, dt, tag=name, bufs=1, name=name)
            ap = src[:] if rearr is None else src[:].rearrange(rearr, p=128)
            nc.sync.dma_start(t[...], ap)
            return t

        wq_sa = wtile("wq_sa", wq_sa_d, [128, ECH, E], "(c p) n -> p c n")
        wk_sa = wtile("wk_sa", wk_sa_d, [128, ECH, E], "(c p) n -> p c n")
        wv_sa = wtile("wv_sa", wv_sa_d, [128, ECH, E], "(c p) n -> p c n")
        wo_sa = wtile("wo_sa", wo_sa_d, [128, ECH, E], "(c p) n -> p c n")
        wq_ca = wtile("wq_ca", wq_ca_d, [128, ECH, E], "(c p) n -> p c n")
        wk_ca = wtile("wk_ca", wk_ca_d, [128, ECH, E], "(c p) n -> p c n")
        wv_ca = wtile("wv_ca", wv_ca_d, [128, ECH, E], "(c p) n -> p c n")
        wo_ca = wtile("wo_ca", wo_ca_d, [128, ECH, E], "(c p) n -> p c n")
        w1 = wtile("w1", w1_d, [128, ECH, F], "(c p) n -> p c n")
        w2 = wtile("w2", w2_d, [128, FCH, E], "(c p) n -> p c n")
        bo_sa = wtile("bo_sa", bo_sa_d, [1, E])
        bo_ca = wtile("bo_ca", bo_ca_d, [1, E])
        b1r = wtile("b1", b1_d, [1, F])
        b2r = wtile("b2", b2_d, [1, E])
        sel_sb = wtile("sel", sel_d, [128, 8, 8])
        selB = wtile("selB", selB_d, [8, 4, 128], dt=F32)
        identb = wtile("identb", identb_d, [128, 128])
        ones_row = wtile("ones", ones_d, [1, E])

        gb = {}
        for k in ("g1", "be1", "g2", "be2", "g3", "be3"):
            row = pools["mrow"].tile([1, E], F32, tag="gbrow", bufs=2, name=k + "r")
            nc.sync.dma_start(row[...], gb_d[k][:])
            bc = wpool.tile([128, E], F32, tag=k, bufs=1, name=k)
            nc.gpsimd.partition_broadcast(bc[:, :], row[:, :])
            gb[k] = bc
        eps = wpool.tile([128, 1], F32, tag="eps", bufs=1, name="eps")
        nc.gpsimd.memset(eps[:, :], 1e-5)

        for e in range(bpc):
            # ---- load x, masks ----
            x_cs = []
            for (t0, tc) in TCH:
                xt = pools["res"].tile([tc, E], F32, name="x_in", tag="res")
                nc.sync.dma_start(xt[:, :], x_d[e, t0:t0 + tc, :])
                x_cs.append(xt)
            pm_row = pools["mrow"].tile([1, T], BF16, name="pm_row")
            nc.sync.dma_start(pm_row[:, :], pm_d[e:e + 1, :])
            pm_bc = pools["mbc"].tile([128, T], BF16, name="pm_bc")
            nc.gpsimd.partition_broadcast(pm_bc[:, :], pm_row[:, :])
            sm_row = pools["mrow"].tile([1, T], BF16, name="sm_row")
            nc.sync.dma_start(sm_row[:, :], sm_d[e:e + 1, :])
            sm_bc = pools["mbc"].tile([128, T], BF16, name="sm_bc")
            nc.gpsimd.partition_broadcast(sm_bc[:, :], sm_row[:, :])

            # ======== self-attention ========
            h_cs = [_layernorm(nc, pools, x_cs[ci][:, :], gb["g1"], gb["be1"], tc, eps)
                    for ci, (t0, tc) in enumerate(TCH)]
            if stages == 10:
                for ci, (t0, tc) in enumerate(TCH):
                    nc.gpsimd.dma_start(out_d[e, t0:t0 + tc, :], h_cs[ci][:, :])
                continue
            hT = _transpose_to(nc, pools, h_cs, identb)
            if stages == 11:
                for ec in range(ECH):
                    nc.gpsimd.dma_start(out_d[e, 0:128, ec * 128:(ec + 1) * 128],
                                        hT[ec][:, 0:128])
                continue
            # masked (pred) transposed h for q,k
            hmT = []
            for ec in range(ECH):
                m = pools["tT"].tile([128, 200], BF16, name="hmT", bufs=5)
                nc.vector.tensor_mul(m[:, 0:200], hT[ec][:, 0:200], pm_bc[:, :])
                hmT.append(m)
            qT = _project_qkT(nc, pools, wq_sa, hmT, "q_sa")
            kT = _project_qkT(nc, pools, wk_sa, hmT, "k_sa")
            v_sb = _project_v(nc, pools, wv_sa, hT, "v_sa")
            if stages == 12:
                for ci, (t0, tc) in enumerate(TCH):
                    nc.gpsimd.dma_start(out_d[e, t0:t0 + tc, :], v_sb[ci][:, :])
                for oc in range(4):
                    nc.gpsimd.dma_start(out_d[e, 0:128, oc * 128:(oc + 1) * 128],
                                        qT[oc][:, 0:128])
                    nc.gpsimd.dma_start(out_d[e, 128:200, oc * 128:(oc + 1) * 128],
                                        kT[oc][0:72, 0:128])
                continue
            x_cs = _attention(nc, pools, qT, kT, v_sb, sel_sb, selB, wo_sa,
                              bo_sa, ones_row, x_cs,
                              out_dump=(stages, out_d, e) if stages in (13, 14) else None)
            if stages in (13, 14):
                continue
            if stages == 1:
                for ci, (t0, tc) in enumerate(TCH):
                    nc.sync.dma_start(out_d[e, t0:t0 + tc, :], x_cs[ci][:, :])
                continue

            # ======== cross-attention ========
            h_cs = [_layernorm(nc, pools, x_cs[ci][:, :], gb["g2"], gb["be2"], tc, eps)
                    for ci, (t0, tc) in enumerate(TCH)]
            h2T = _transpose_to(nc, pools, h_cs, identb)
            # memory: natural load then PE transpose + src-mask multiply
            m_cs = []
            for (t0, tc) in TCH:
                mt = pools["h"].tile([tc, E], BF16, name="m_nat", tag="m_nat",
                                     bufs=3)
                nc.sync.dma_start(mt[:, :], mem_d[e, t0:t0 + tc, :])
                m_cs.append(mt)
            memT = []
            mT = _transpose_to(nc, pools, m_cs, identb)
            for ec in range(ECH):
                mm = pools["tT"].tile([128, 200], BF16, name="memTm", bufs=5)
                nc.vector.tensor_mul(mm[:, 0:200], mT[ec][:, 0:200], sm_bc[:, :])
                memT.append(mm)
            qT = _project_qkT(nc, pools, wq_ca, h2T, "q_ca")
            kT = _project_qkT(nc, pools, wk_ca, memT, "k_ca")
            v_sb = _project_v(nc, pools, wv_ca, h2T, "v_ca")
            x_cs = _attention(nc, pools, qT, kT, v_sb, sel_sb, selB, wo_ca,
                              bo_ca, ones_row, x_cs)
            if stages == 2:
                for ci, (t0, tc) in enumerate(TCH):
                    nc.sync.dma_start(out_d[e, t0:t0 + tc, :], x_cs[ci][:, :])
                continue

            # ======== feed-forward ========
            h_cs = [_layernorm(nc, pools, x_cs[ci][:, :], gb["g3"], gb["be3"], tc, eps)
                    for ci, (t0, tc) in enumerate(TCH)]
            h3T = _transpose_to(nc, pools, h_cs, identb)
            rT = []
            for fcp in range(FCH // 2):
                zps = pools["ps_st"].tile([128, 2, 200], F32, name="z_ps", tag="st")
                for fl in range(2):
                    fc = 2 * fcp + fl
                    for ec in range(ECH):
                        nc.tensor.matmul(
                            zps[:, fl, :], w1[:, ec, fc * 128:(fc + 1) * 128],
                            h3T[ec][:, 0:200], start=(ec == 0), stop=False)
                    nc.tensor.matmul(
                        zps[:, fl, :], b1r[0:1, fc * 128:(fc + 1) * 128],
                        ones_row[0:1, 0:200], start=False, stop=True)
                r = pools["rT"].tile([128, 2, 200], BF16, name="r")
                nc.scalar.activation(r[:, :, :], zps[:, :, :], AF.Relu)
                rT.append(r)
            for ci, (t0, tc) in enumerate(TCH):
                yps = pools["ps_proj"].tile([tc, E], F32, name="y_ps", tag="pp")
                for fcp in range(FCH // 2):
                    for fl in range(2):
                        fc = 2 * fcp + fl
                        nc.tensor.matmul(yps[:, :], rT[fcp][:, fl, t0:t0 + tc],
                                         w2[:, fc, :], start=(fc == 0), stop=False)
                nc.tensor.matmul(yps[:, :], ones_row[0:1, 0:tc], b2r[0:1, :],
                                 start=False, stop=True)
                yout = pools["res"].tile([tc, E], F32, name="yout", tag="res")
                nc.vector.tensor_add(yout[:, :], yps[:, :], x_cs[ci][:, :])
                nc.sync.dma_start(out_d[e, t0:t0 + tc, :], yout[:, :])

    nc.compile()
    return nc


def _host_prep(inputs, bpc, core):
    """Build the in_map for one core."""
    s = slice(core * bpc, (core + 1) * bpc)

    def rearr(w):  # (H, E, D) -> [E, H*D]
        return np.ascontiguousarray(
            np.transpose(np.asarray(w, np.float32), (1, 0, 2)).reshape(E, E)
        ).astype(NPBF16)

    def b16(a):
        return np.ascontiguousarray(np.asarray(a, np.float32)).astype(NPBF16)

    def f32c(a):
        return np.ascontiguousarray(np.asarray(a, np.float32))

    return {
        "x": f32c(inputs["idx"][s]),
        "mem": b16(inputs["memory"][s]),
        "pm": b16(inputs["pred_mask"][s] != 0),
        "sm": b16(inputs["src_mask"][s] != 0),
        "wq_sa": rearr(inputs["sa_wq"]), "wk_sa": rearr(inputs["sa_wk"]),
        "wv_sa": rearr(inputs["sa_wv"]),
        "wo_sa": b16(inputs["sa_wo"]), "bo_sa": b16(inputs["sa_bo"]).reshape(1, E),
        "wq_ca": rearr(inputs["ca_wq"]), "wk_ca": rearr(inputs["ca_wk"]),
        "wv_ca": rearr(inputs["ca_wv"]),
        "wo_ca": b16(inputs["ca_wo"]), "bo_ca": b16(inputs["ca_bo"]).reshape(1, E),
        "w1": b16(inputs["f_w1"]), "b1": b16(inputs["f_b1"]).reshape(1, F),
        "w2": b16(inputs["f_w2"]), "b2": b16(inputs["f_b2"]).reshape(1, E),
        "g1": f32c(inputs["ln1_g"]).reshape(1, E),
        "be1": f32c(inputs["ln1_b"]).reshape(1, E),
        "g2": f32c(inputs["ln2_g"]).reshape(1, E),
        "be2": f32c(inputs["ln2_b"]).reshape(1, E),
        "g3": f32c(inputs["ln3_g"]).reshape(1, E),
        "be3": f32c(inputs["ln3_b"]).reshape(1, E),
    }


def get_program(bpc):
    if bpc not in _programs:
        _programs[bpc] = _build(bpc)
    return _programs[bpc]


def kernel(**inputs) -> np.ndarray:
    bpc = B // NCORES
    nc = get_program(bpc)
    in_maps = [_host_prep(inputs, bpc, c) for c in range(NCORES)]
    res = run_bass_kernel_spmd(nc, in_maps, core_ids=list(range(NCORES)))
    out = np.concatenate([res.results[c]["out"] for c in range(NCORES)], axis=0)
    return out.astype(np.float32)


# revision 18
# speedup vs baseline: 1.7576x; 1.3322x over previous
"""Trainium2 Bass kernel for a single transformer decoder layer.

Reference semantics (B=64, T=200, E=512, H=8, D=64):
  x += SelfAttn(LN1(x))   (q,k row-masked by pred_mask, causal)
  x += CrossAttn(LN2(x))  (k from raw memory row-masked by src_mask,
                           v from LN2(x) (!), causal)
  x += FFN(LN3(x))        (512 -> 2048 -> relu -> 512)

Sharding: data-parallel over batch, 8 elems per NeuronCore, no collectives.

Layout strategy (per core, batch elems processed in PAIRS):
  - residual stream x kept NATURAL [t_chunk<=128, 512] in fp32
  - LN via bn_stats/bn_aggr + two fused scalar_tensor_tensor ops
  - activations transposed to [E, 2*T] pair tiles via PE is_transpose
    matmuls (keeps PE warm), DVE drains the PSUM
  - Q,K projected transposed [H*D, 2*T] with weight stationaries, N=400
  - scores computed TRANSPOSED  ST[s, t] = K Q^T  per head per elem,
    2 heads per PSUM bank; exp on ACT (no max subtraction -- scores are
    O(1)); causal mask applied post-exp via gpsimd.affine_select(fill=0)
  - matmul operands must sit at SBUF base partition 0 (row-group-64
    operands crash the device), so odd heads read DMA-shifted copies
  - softmax denominators via one-hot-column matmuls into [8,T] PSUM;
    1/d via reciprocal_approx_fast, broadcast to head halves by a
    one-hot matmul, multiplied into O^T on DVE
  - AV gives O transposed directly (lhsT = V natural slices)
  - biases enter PSUM via rank-1 (K=1) matmuls; FFN b1 rides the
    relu activation bias (per-partition in the transposed layout)
"""

import numpy as np
import ml_dtypes
from contextlib import ExitStack

import concourse.bass as bass
import concourse.bacc as bacc
import concourse.tile as tile
from concourse import mybir
from concourse.bass_utils import run_bass_kernel_spmd

B, T, E, H, Dh, F = 64, 200, 512, 8, 64, 2048
NCORES = 8
SCALE = float(E) ** -0.5
F32 = mybir.dt.float32
BF16 = mybir.dt.bfloat16
AL = mybir.AluOpType
AF = mybir.ActivationFunctionType
TCH = [(0, 128), (128, 72)]  # token chunks (t0, tc)
ECH = E // 128  # 4
FCH = F // 128  # 16
NPBF16 = ml_dtypes.bfloat16

_programs = {}


def _layernorm(nc, pools, x_c, G, Bt, tc, eps):
    """x_c: [tc,512] f32 natural -> h_c [tc,512] bf16."""
    st6 = pools["small"].tile([tc, 6], F32, name="st6")
    nc.vector.bn_stats(st6[:, :], x_c)
    mv = pools["small"].tile([tc, 2], F32, name="mv")
    nc.vector.bn_aggr(mv[:, :], st6[:, :])
    std = pools["small"].tile([tc, 1], F32, name="std")
    nc.scalar.activation(std[:, :], mv[:, 1:2], AF.Sqrt, bias=eps[0:tc, 0:1])
    rstd = pools["small"].tile([tc, 1], F32, name="rstd")
    nc.vector.reciprocal(rstd[:, :], std[:, :])
    t1 = pools["lnt"].tile([tc, E], F32, name="t1")
    nc.vector.scalar_tensor_tensor(
        t1[:, :], x_c, mv[:, 0:1], G[0:tc, :], op0=AL.subtract, op1=AL.mult)
    h_c = pools["h"].tile([tc, E], BF16, name="h_c", tag="h_c", bufs=6)
    nc.vector.scalar_tensor_tensor(
        h_c[:, :], t1[:, :], rstd[:, 0:1], Bt[0:tc, :], op0=AL.mult, op1=AL.add)
    return h_c


def _transpose_pair(nc, pools, h_cs_pair, ident):
    """h_cs_pair: list of 2 elems x 2 chunks of [tc,512] bf16 natural ->
    hT[ec] [128, 400] bf16 pair tiles via PE transposes."""
    hT = []
    for ec in range(ECH):
        t = pools["tT"].tile([128, 2 * T], BF16, name="hT", bufs=6)
        for el in range(2):
            for ci, (t0, tc) in enumerate(TCH):
                ps = pools["ps_proj"].tile([128, tc], BF16, name="t_ps", tag="pp")
                nc.tensor.transpose(
                    ps[:, :], h_cs_pair[el][ci][0:tc, ec * 128:(ec + 1) * 128],
                    ident[0:tc, 0:tc])
                nc.vector.tensor_copy(t[:, el * T + t0:el * T + t0 + tc], ps[:, :])
        hT.append(t)
    return hT


def _project_qkT(nc, pools, w_sb, rhs_T, name):
    """[128, 400] bf16 pair chunks of (W^T h)^T, plus base-partition-0
    copies of rows 64:128 (odd heads must read from partition 0)."""
    out, hi = [], []
    for oc in range(4):
        ps = pools["ps_proj"].tile([128, 2 * T], F32, name=f"{name}_ps", tag="pp")
        for ec in range(ECH):
            nc.tensor.matmul(ps[:, :], w_sb[:, ec, oc * 128:(oc + 1) * 128],
                             rhs_T[ec][:, :], start=(ec == 0), stop=(ec == 3))
        qk = "q" if name.startswith("q") else "k"
        sb = pools["qkt"].tile([128, 2 * T], BF16, name=f"{name}_sb", tag=qk, bufs=5)
        nc.vector.tensor_copy(sb[:, :], ps[:, :])
        hb = pools["qkt"].tile([64, 2 * T], BF16, name=f"{name}_hi", tag="hi",
                               bufs=10)
        nc.sync.dma_start(hb[:, :], sb[64:128, :])
        out.append(sb)
        hi.append(hb)
    return out, hi


def _project_v(nc, pools, wv_sb, hT, off, name):
    """v natural [tc, 512] bf16 tiles for ONE elem (lhsT = hT pair slices)."""
    out = []
    for (t0, tc) in TCH:
        ps = pools["ps_proj"].tile([tc, E], F32, name=f"{name}_ps", tag="pp")
        for ec in range(ECH):
            nc.tensor.matmul(ps[:, :], hT[ec][:, off + t0:off + t0 + tc],
                             wv_sb[:, ec, :], start=(ec == 0), stop=(ec == 3))
        sb = pools["v"].tile([tc, E], BF16, name=f"{name}_sb", tag="v", bufs=6)
        nc.scalar.copy(sb[:, :], ps[:, :])
        out.append(sb)
    return out


def _attention(nc, pools, qkt, v_sb, sel_sb, selB, wo_sb, bo_row, ones_row,
               x_cs, off):
    """Causal attention for ONE elem (token cols off:off+200 of the pair
    tiles) + output projection + bias + residual."""
    (qT_lo, qT_hi), (kT_lo, kT_hi) = qkt
    e0m, e1m = [], []
    # pass A: scores (transposed), exp, causal select; 2 heads per psum bank
    for oc in range(4):
        st0 = pools["ps_st"].tile([128, 2, 200], F32, name="st0", tag="st")
        st1 = pools["ps_st"].tile([72, 2, 72], F32, name="st1", tag="st")
        for hl in range(2):
            qh = (qT_lo, qT_hi)[hl][oc][0:64, off:off + 200]
            kh = (kT_lo, kT_hi)[hl][oc][0:64, off:off + 200]
            nc.tensor.matmul(st0[:, hl, :], kh[:, 0:128], qh)
            nc.tensor.matmul(st1[:, hl, :], kh[:, 128:200], qh[:, 128:200])
        e0 = pools["e0"].tile([128, 2, 200], BF16, name="e0", bufs=3)
        nc.scalar.activation(e0[:, :, :], st0[:, :, :], AF.Exp, scale=SCALE)
        e1 = pools["e1"].tile([72, 2, 72], BF16, name="e1", bufs=3)
        nc.scalar.activation(e1[:, :, :], st1[:, :, :], AF.Exp, scale=SCALE)
        # causal: keep where t - s >= 0 (iota = -p + t), else 0
        e0x = pools["e0"].tile([128, 2, 200], BF16, name="e0x", bufs=5)
        nc.gpsimd.affine_select(
            e0x[:, :, :], e0[:, :, :], pattern=[[0, 2], [1, 200]],
            compare_op=AL.is_ge, fill=0.0, base=0, channel_multiplier=-1)
        e1x = pools["e1"].tile([72, 2, 72], BF16, name="e1x", bufs=5)
        nc.gpsimd.affine_select(
            e1x[:, :, :], e1[:, :, :], pattern=[[0, 2], [1, 72]],
            compare_op=AL.is_ge, fill=0.0, base=0, channel_multiplier=-1)
        e0m.append(e0x)
        e1m.append(e1x)
    # pass B: denominators d[h, t] = sum_s exp -- one-hot stationaries
    dT = pools["ps_d"].tile([8, 200], F32, name="dT")
    for oc in range(4):
        for hl in range(2):
            h = 2 * oc + hl
            nc.tensor.matmul(dT[:, 0:200], sel_sb[0:128, h, :], e0m[oc][:, hl, :],
                             start=(h == 0), stop=False, skip_group_check=True)
            nc.tensor.matmul(dT[:, 128:200], sel_sb[0:72, h, :], e1m[oc][:, hl, :],
                             start=False, stop=(h == 7), skip_group_check=True)
    dt_sb = pools["small"].tile([8, 200], F32, name="dt_sb")
    nc.vector.tensor_copy(dt_sb[:, :], dT[:, :])
    dinvT = pools["small"].tile([8, 200], F32, name="dinvT")
    nc.vector.reciprocal_approx_fast(dinvT[:, :], dt_sb[:, :])
    # pass C: O^T = V^T @ E^T, normalized by 1/d broadcast to head halves
    oT_sb = []
    for oc in range(4):
        dbc_ps = pools["ps_ot"].tile([128, 200], F32, name="dbc_ps", tag="ot_ps")
        nc.tensor.matmul(dbc_ps[:, :], selB[0:8, oc, :], dinvT[:, :])
        dbc = pools["dbc"].tile([128, 200], F32, name="dbc")
        nc.vector.tensor_copy(dbc[:, :], dbc_ps[:, :])
        ot_ps = pools["ps_ot"].tile([128, 200], F32, name="ot_ps", tag="ot_ps")
        for hl in range(2):
            h = 2 * oc + hl
            hp = hl * 64
            nc.tensor.matmul(ot_ps[hp:hp + 64, 0:200],
                             v_sb[0][0:128, h * 64:(h + 1) * 64],
                             e0m[oc][:, hl, :], start=True, stop=False,
                             skip_group_check=True)
            nc.tensor.matmul(ot_ps[hp:hp + 64, 128:200],
                             v_sb[1][0:72, h * 64:(h + 1) * 64],
                             e1m[oc][:, hl, :], start=False, stop=True,
                             skip_group_check=True)
        ot = pools["ot"].tile([128, 200], BF16, name="ot", bufs=6)
        nc.vector.tensor_mul(ot[:, :], ot_ps[:, :], dbc[:, :])
        oT_sb.append(ot)
    # output projection (natural) + bias via rank-1 matmul + residual
    new_x = []
    for ci, (t0, tc) in enumerate(TCH):
        ps = pools["ps_proj"].tile([tc, E], F32, name="proj_ps", tag="pp")
        for hc in range(4):
            nc.tensor.matmul(ps[:, :], oT_sb[hc][:, t0:t0 + tc],
                             wo_sb[:, hc, :], start=(hc == 0), stop=False)
        nc.tensor.matmul(ps[:, :], ones_row[0:1, 0:tc], bo_row[0:1, :],
                         start=False, stop=True)
        xn = pools["res"].tile([tc, E], F32, name="xn", tag="res")
        nc.vector.tensor_add(xn[:, :], ps[:, :], x_cs[ci])
        new_x.append(xn)
    return new_x


def _build(bpc, stages=3):
    nc = bacc.Bacc("TRN2", target_bir_lowering=False, debug=False,
                   enable_asserts=False, num_devices=NCORES)
    dram = {}

    def din(name, shape, dt):
        h = nc.dram_tensor(name, list(shape), dt, kind="ExternalInput")
        dram[name] = h
        return h

    x_d = din("x", (bpc, T, E), F32)
    mem_d = din("mem", (bpc, T, E), BF16)
    pm_d = din("pm", (bpc, T), BF16)
    sm_d = din("sm", (bpc, T), BF16)
    wq_sa_d = din("wq_sa", (E, E), BF16)
    wk_sa_d = din("wk_sa", (E, E), BF16)
    wv_sa_d = din("wv_sa", (E, E), BF16)
    wo_sa_d = din("wo_sa", (E, E), BF16)
    bo_sa_d = din("bo_sa", (1, E), BF16)
    wq_ca_d = din("wq_ca", (E, E), BF16)
    wk_ca_d = din("wk_ca", (E, E), BF16)
    wv_ca_d = din("wv_ca", (E, E), BF16)
    wo_ca_d = din("wo_ca", (E, E), BF16)
    bo_ca_d = din("bo_ca", (1, E), BF16)
    w1_d = din("w1", (E, F), BF16)
    b1_d = din("b1", (1, F), BF16)
    w2_d = din("w2", (F, E), BF16)
    b2_d = din("b2", (1, E), BF16)
    gb_d = {k: din(k, (1, E), F32)
            for k in ("g1", "be1", "g2", "be2", "g3", "be3")}
    out_d = nc.dram_tensor("out", [bpc, T, E], F32, kind="ExternalOutput")

    sel_np = np.zeros((128, 8, 8), dtype=NPBF16)
    for h in range(8):
        sel_np[:, h, h] = 1
    sel_d = nc.inline_tensor(sel_np, name="selc")
    ones_d = nc.inline_tensor(np.ones((1, E), dtype=NPBF16), name="onesc")
    selB_np = np.zeros((8, 4, 128), dtype=np.float32)
    for oc in range(4):
        selB_np[2 * oc, oc, 0:64] = 1
        selB_np[2 * oc + 1, oc, 64:128] = 1
    selB_d = nc.inline_tensor(selB_np, name="selBc")
    identb_d = nc.inline_tensor(np.eye(128, dtype=NPBF16), name="identbc")

    with tile.TileContext(nc) as tcx, ExitStack() as ctx:
        pools = {}

        def pool(name, bufs, space="SBUF"):
            pools[name] = ctx.enter_context(
                tcx.tile_pool(name=name, bufs=bufs, space=space))
            return pools[name]

        wpool = pool("w", 1)
        pool("small", 6)
        pool("lnt", 3)
        pool("h", 6)
        pool("tT", 5)
        pool("qkt", 5)
        pool("v", 5)
        pool("e0", 3)
        pool("e1", 3)
        pool("ot", 6)
        pool("dbc", 3)
        pool("res", 12)
        pool("rT", 17)
        pool("mrow", 3)
        pool("mbc", 5)
        pool("ps_proj", 3, space="PSUM")
        pool("ps_st", 2, space="PSUM")
        pool("ps_d", 1, space="PSUM")
        pool("ps_ot", 2, space="PSUM")

        def wtile(name, src, shape, rearr=None, dt=BF16):
            t = wpool.tile(shape, dt, tag=name, bufs=1, name=name)
            ap = src[:] if rearr is None else src[:].rearrange(rearr, p=128)
            nc.sync.dma_start(t[...], ap)
            return t

        wq_sa = wtile("wq_sa", wq_sa_d, [128, ECH, E], "(c p) n -> p c n")
        wk_sa = wtile("wk_sa", wk_sa_d, [128, ECH, E], "(c p) n -> p c n")
        wv_sa = wtile("wv_sa", wv_sa_d, [128, ECH, E], "(c p) n -> p c n")
        wo_sa = wtile("wo_sa", wo_sa_d, [128, ECH, E], "(c p) n -> p c n")
        wq_ca = wtile("wq_ca", wq_ca_d, [128, ECH, E], "(c p) n -> p c n")
        wk_ca = wtile("wk_ca", wk_ca_d, [128, ECH, E], "(c p) n -> p c n")
        wv_ca = wtile("wv_ca", wv_ca_d, [128, ECH, E], "(c p) n -> p c n")
        wo_ca = wtile("wo_ca", wo_ca_d, [128, ECH, E], "(c p) n -> p c n")
        w1 = wtile("w1", w1_d, [128, ECH, F], "(c p) n -> p c n")
        w2 = wtile("w2", w2_d, [128, FCH, E], "(c p) n -> p c n")
        bo_sa = wtile("bo_sa", bo_sa_d, [1, E])
        bo_ca = wtile("bo_ca", bo_ca_d, [1, E])
        b2r = wtile("b2", b2_d, [1, E])
        sel_sb = wtile("sel", sel_d, [128, 8, 8])
        selB = wtile("selB", selB_d, [8, 4, 128], dt=F32)
        identb = wtile("identb", identb_d, [128, 128])
        ones_row = wtile("ones", ones_d, [1, E])
        # f_b1 in column layout [128, 16] for the relu per-partition bias
        b1c = wpool.tile([128, FCH], F32, tag="b1c", bufs=1, name="b1c")
        b1cb = wpool.tile([128, FCH], BF16, tag="b1cb", bufs=1, name="b1cb")
        nc.sync.dma_start(b1cb[...], b1_d[:].rearrange("o (c p) -> p (o c)", p=128))
        nc.vector.tensor_copy(b1c[:, :], b1cb[:, :])

        gb = {}
        for k in ("g1", "be1", "g2", "be2", "g3", "be3"):
            row = pools["mrow"].tile([1, E], F32, tag="gbrow", bufs=1,
                                     name=k + "r")
            nc.sync.dma_start(row[...], gb_d[k][:])
            bc = wpool.tile([128, E], F32, tag=k, bufs=1, name=k)
            nc.gpsimd.partition_broadcast(bc[:, :], row[:, :])
            gb[k] = bc
        eps = wpool.tile([128, 1], F32, tag="eps", bufs=1, name="eps")
        nc.gpsimd.memset(eps[:, :], 1e-5)

        for pr in range(bpc // 2):
            els = (2 * pr, 2 * pr + 1)
            # ---- load x and masks for both elems ----
            x_el = []
            pm2 = pools["mbc"].tile([128, 2 * T], BF16, name="pm2")
            sm2 = pools["mbc"].tile([128, 2 * T], BF16, name="sm2")
            for el, e in enumerate(els):
                x_cs = []
                for (t0, tc) in TCH:
                    xt = pools["res"].tile([tc, E], F32, name="x_in", tag="res")
                    nc.sync.dma_start(xt[:, :], x_d[e, t0:t0 + tc, :])
                    x_cs.append(xt)
                x_el.append(x_cs)
                pm_row = pools["mrow"].tile([1, T], BF16, name="pm_row", bufs=2)
                nc.sync.dma_start(pm_row[:, :], pm_d[e:e + 1, :])
                nc.gpsimd.partition_broadcast(pm2[:, el * T:(el + 1) * T],
                                              pm_row[:, :])
                sm_row = pools["mrow"].tile([1, T], BF16, name="sm_row", bufs=2)
                nc.sync.dma_start(sm_row[:, :], sm_d[e:e + 1, :])
                nc.gpsimd.partition_broadcast(sm2[:, el * T:(el + 1) * T],
                                              sm_row[:, :])

            # ======== self-attention ========
            h_pair = [[_layernorm(nc, pools, x_el[el][ci][:, :], gb["g1"],
                                  gb["be1"], tc, eps)
                       for ci, (t0, tc) in enumerate(TCH)] for el in range(2)]
            hT = _transpose_pair(nc, pools, h_pair, identb)
            hmT = []
            for ec in range(ECH):
                m = pools["tT"].tile([128, 2 * T], BF16, name="hmT", bufs=5)
                nc.vector.tensor_mul(m[:, :], hT[ec][:, :], pm2[:, :])
                hmT.append(m)
            qT = _project_qkT(nc, pools, wq_sa, hmT, "q_sa")
            kT = _project_qkT(nc, pools, wk_sa, hmT, "k_sa")
            for el in range(2):
                v_sb = _project_v(nc, pools, wv_sa, hT, el * T, "v_sa")
                x_el[el] = _attention(nc, pools, (qT, kT), v_sb, sel_sb, selB,
                                      wo_sa, bo_sa, ones_row, x_el[el], el * T)
            if stages == 1:
                for el, e in enumerate(els):
                    for ci, (t0, tc) in enumerate(TCH):
                        nc.sync.dma_start(out_d[e, t0:t0 + tc, :],
                                          x_el[el][ci][:, :])
                continue

            # ======== cross-attention ========
            h_pair = [[_layernorm(nc, pools, x_el[el][ci][:, :], gb["g2"],
                                  gb["be2"], tc, eps)
                       for ci, (t0, tc) in enumerate(TCH)] for el in range(2)]
            h2T = _transpose_pair(nc, pools, h_pair, identb)
            m_pair = []
            for el, e in enumerate(els):
                m_cs = []
                for (t0, tc) in TCH:
                    mt = pools["h"].tile([tc, E], BF16, name="m_nat",
                                         tag="m_nat", bufs=6)
                    nc.sync.dma_start(mt[:, :], mem_d[e, t0:t0 + tc, :])
                    m_cs.append(mt)
                m_pair.append(m_cs)
            mT = _transpose_pair(nc, pools, m_pair, identb)
            memT = []
            for ec in range(ECH):
                mm = pools["tT"].tile([128, 2 * T], BF16, name="memTm", bufs=5)
                nc.vector.tensor_mul(mm[:, :], mT[ec][:, :], sm2[:, :])
                memT.append(mm)
            qT = _project_qkT(nc, pools, wq_ca, h2T, "q_ca")
            kT = _project_qkT(nc, pools, wk_ca, memT, "k_ca")
            for el in range(2):
                v_sb = _project_v(nc, pools, wv_ca, h2T, el * T, "v_ca")
                x_el[el] = _attention(nc, pools, (qT, kT), v_sb, sel_sb, selB,
                                      wo_ca, bo_ca, ones_row, x_el[el], el * T)
            if stages == 2:
                for el, e in enumerate(els):
                    for ci, (t0, tc) in enumerate(TCH):
                        nc.sync.dma_start(out_d[e, t0:t0 + tc, :],
                                          x_el[el][ci][:, :])
                continue

            # ======== feed-forward ========
            h_pair = [[_layernorm(nc, pools, x_el[el][ci][:, :], gb["g3"],
                                  gb["be3"], tc, eps)
                       for ci, (t0, tc) in enumerate(TCH)] for el in range(2)]
            h3T = _transpose_pair(nc, pools, h_pair, identb)
            rT = []
            for fc in range(FCH):
                zps = pools["ps_st"].tile([128, 2 * T], F32, name="z_ps",
                                          tag="st")
                for ec in range(ECH):
                    nc.tensor.matmul(zps[:, :],
                                     w1[:, ec, fc * 128:(fc + 1) * 128],
                                     h3T[ec][:, :], start=(ec == 0),
                                     stop=(ec == 3))
                r = pools["rT"].tile([128, 2 * T], BF16, name="r")
                nc.scalar.activation(r[:, :], zps[:, :], AF.Relu,
                                     bias=b1c[:, fc:fc + 1])
                rT.append(r)
            for el, e in enumerate(els):
                for ci, (t0, tc) in enumerate(TCH):
                    yps = pools["ps_proj"].tile([tc, E], F32, name="y_ps",
                                                tag="pp")
                    for fc in range(FCH):
                        nc.tensor.matmul(yps[:, :],
                                         rT[fc][:, el * T + t0:el * T + t0 + tc],
                                         w2[:, fc, :], start=(fc == 0),
                                         stop=False)
                    nc.tensor.matmul(yps[:, :], ones_row[0:1, 0:tc],
                                     b2r[0:1, :], start=False, stop=True)
                    yout = pools["res"].tile([tc, E], F32, name="yout",
                                             tag="res")
                    nc.vector.tensor_add(yout[:, :], yps[:, :],
                                         x_el[el][ci][:, :])
                    nc.sync.dma_start(out_d[e, t0:t0 + tc, :], yout[:, :])

    nc.compile()
    return nc


def _host_prep(inputs, bpc, core):
    """Build the in_map for one core."""
    s = slice(core * bpc, (core + 1) * bpc)

    def rearr(w):  # (H, E, D) -> [E, H*D]
        return np.ascontiguousarray(
            np.transpose(np.asarray(w, np.float32), (1, 0, 2)).reshape(E, E)
        ).astype(NPBF16)

    def b16(a):
        return np.ascontiguousarray(np.asarray(a, np.float32)).astype(NPBF16)

    def f32c(a):
        return np.ascontiguousarray(np.asarray(a, np.float32))

    return {
        "x": f32c(inputs["idx"][s]),
        "mem": b16(inputs["memory"][s]),
        "pm": b16(inputs["pred_mask"][s] != 0),
        "sm": b16(inputs["src_mask"][s] != 0),
        "wq_sa": rearr(inputs["sa_wq"]), "wk_sa": rearr(inputs["sa_wk"]),
        "wv_sa": rearr(inputs["sa_wv"]),
        "wo_sa": b16(inputs["sa_wo"]), "bo_sa": b16(inputs["sa_bo"]).reshape(1, E),
        "wq_ca": rearr(inputs["ca_wq"]), "wk_ca": rearr(inputs["ca_wk"]),
        "wv_ca": rearr(inputs["ca_wv"]),
        "wo_ca": b16(inputs["ca_wo"]), "bo_ca": b16(inputs["ca_bo"]).reshape(1, E),
        "w1": b16(inputs["f_w1"]), "b1": b16(inputs["f_b1"]).reshape(1, F),
        "w2": b16(inputs["f_w2"]), "b2": b16(inputs["f_b2"]).reshape(1, E),
        "g1": f32c(inputs["ln1_g"]).reshape(1, E),
        "be1": f32c(inputs["ln1_b"]).reshape(1, E),
        "g2": f32c(inputs["ln2_g"]).reshape(1, E),
        "be2": f32c(inputs["ln2_b"]).reshape(1, E),
        "g3": f32c(inputs["ln3_g"]).reshape(1, E),
        "be3": f32c(inputs["ln3_b"]).reshape(1, E),
    }


def get_program(bpc):
    if bpc not in _programs:
        _programs[bpc] = _build(bpc)
    return _programs[bpc]


def kernel(**inputs) -> np.ndarray:
    bpc = B // NCORES
    nc = get_program(bpc)
    in_maps = [_host_prep(inputs, bpc, c) for c in range(NCORES)]
    res = run_bass_kernel_spmd(nc, in_maps, core_ids=list(range(NCORES)))
    out = np.concatenate([res.results[c]["out"] for c in range(NCORES)], axis=0)
    return out.astype(np.float32)
